# revision 1
# baseline (speedup 1.0000x reference)
"""Trainium2 Bass kernel for ternary-quantized attention (BitNet-style).

Host contract: kernel(x, w_qkv) -> [16,1025,768] fp32.
Shards B=16 over 8 cores (2 batches/core), replicates the ternary weight.

Math (validated vs fp32 reference at ~0.8% rel err):
  - w ternarized on host to {-1,0,1}; s_w folded out (scale-invariant l1norm).
  - qkv = x_hi@wt + x_lo@wt  (split-bf16, fp32 PSUM accumulate).
  - q/k/v quantize: u = t / (l1_row * s_const), s_const = 1/64 + 1e-5;
    ternary = sign(bf16(u+192) - 192)   [bf16 write rounds to integer, RNE]
  - attn_int = q_q @ k_q^T  (exact ternary bf16 matmuls, fp32 accum)
  - per-(b,h) scale: t = mean|attn_int| + EPS/(scale*s_const^2); rho = 1/t
  - attn_q = sign(bf16(attn_int*rho + 192) - 192)
  - out = (attn_q @ v_q) * (scale * s_const^3 * t)
"""
import sys, os
sys.path.insert(0, "/opt/trn_rl_repo")
import numpy as np
import ml_dtypes
from contextlib import ExitStack

import concourse.bass as bass
import concourse.tile as tile
from concourse import bacc
from concourse import mybir
from concourse.bass_utils import run_bass_kernel_spmd

EPS = 1e-5
B, N, C, H, D = 16, 1025, 768, 12, 64
BPC = B // 8  # batches per core
SCALE = float(D) ** -0.5
S_CONST = np.float32(1.0 / D) + np.float32(EPS)
C_EPS = np.float32(EPS) / (np.float32(SCALE) * S_CONST * S_CONST)
KAPPA = np.float32(SCALE) * S_CONST * S_CONST * S_CONST
M192 = 192.0

F32 = mybir.dt.float32
BF16 = mybir.dt.bfloat16

N_TILES = [(0, 512), (512, 512), (1024, 1)]
M_CHUNKS = [(i * 128, 128) for i in range(8)] + [(1024, 1)]
AS_STRIDE = 1026  # attn_sb per-m-chunk column stride (even, >=1025)


def build_nc():
    nc = bacc.Bacc("TRN2", target_bir_lowering=False, debug=False,
                   enable_asserts=False, num_devices=8)
    # register const APs used as activation biases
    for val in (-M192,):
        t = nc.alloc_sbuf_tensor(f"const-f32-{val}", [128, 1], F32)
        nc.gpsimd.memset(t.ap(), val)
        nc.const_aps.aps[(F32, val)] = t.ap()
    nc.all_engine_barrier()
    x_d = nc.dram_tensor("x_sh", [BPC, C, N], F32, kind="ExternalInput").ap()
    wt_d = nc.dram_tensor("wt_bf", [C, 3 * C], BF16, kind="ExternalInput").ap()
    id_d = nc.dram_tensor("ident", [128, 128], BF16, kind="ExternalInput").ap()
    on_d = nc.dram_tensor("ones128", [128, 1], F32, kind="ExternalInput").ap()
    y_d = nc.dram_tensor("y_sh", [BPC, C, N], F32, kind="ExternalOutput").ap()

    with tile.TileContext(nc) as tc, ExitStack() as ctx:
        # ---- persistent pools ----
        const_p = ctx.enter_context(tc.tile_pool(name="consts", bufs=1))
        wt_p = ctx.enter_context(tc.tile_pool(name="wt", bufs=6))
        qt_p = ctx.enter_context(tc.tile_pool(name="qt", bufs=6 * BPC))
        kt_p = ctx.enter_context(tc.tile_pool(name="kt", bufs=6 * BPC))
        vq_p = ctx.enter_context(tc.tile_pool(name="vq", bufs=BPC))

        ident = const_p.tile([128, 128], BF16, tag="ident")
        nc.gpsimd.dma_start(ident[:], id_d)
        ones128 = const_p.tile([128, 1], F32, tag="ones")
        nc.gpsimd.dma_start(ones128[:], on_d)

        wt = []
        for c in range(6):
            t = wt_p.tile([128, 3 * C], BF16, tag="wt")
            nc.gpsimd.dma_start(t[:], wt_d[c * 128:(c + 1) * 128, :])
            wt.append(t)

        # per-batch persistent ternary stores
        qT = [[qt_p.tile([128, N], BF16, tag="qt", name=f"qT_{b}_{j}") for j in range(6)] for b in range(BPC)]
        kT = [[kt_p.tile([128, N], BF16, tag="kt", name=f"kT_{b}_{j}") for j in range(6)] for b in range(BPC)]
        vq = [vq_p.tile([128, 9 * C], BF16, tag="vq", name=f"vq_{b}") for b in range(BPC)]

        # ================= PHASE A: qkv + quantize + transpose =================
        import os as _os
        _PH = _os.environ.get("KERNEL_PHASE", "full")
        with tc.tile_pool(name="xs", bufs=2) as xs_p, \
             tc.tile_pool(name="xhl", bufs=2 * 6) as xhl_p, \
             tc.tile_pool(name="qkvsb", bufs=2) as qkvsb_p, \
             tc.tile_pool(name="small_a", bufs=4) as small_p, \
             tc.tile_pool(name="y192", bufs=2) as y192_p, \
             tc.tile_pool(name="qkq", bufs=2) as qkq_p, \
             tc.tile_pool(name="ps_qkv", bufs=3, space="PSUM") as ps_qkv, \
             tc.tile_pool(name="ps_tr", bufs=3, space="PSUM") as ps_tr:
            for b in (range(BPC) if _PH in ("full", "A") else []):
                # load xT chunks and split hi/lo
                xhi, xlo = [], []
                for c in range(6):
                    xf = xs_p.tile([128, N], F32, tag="xs")
                    nc.gpsimd.dma_start(xf[:], x_d[b, c * 128:(c + 1) * 128, :])
                    th = xhl_p.tile([128, N], BF16, tag="xhl")
                    nc.vector.tensor_copy(th[:], xf[:])
                    tl = xhl_p.tile([128, N], BF16, tag="xhl")
                    nc.vector.scalar_tensor_tensor(
                        tl[:], xf[:], 1.0, th[:],
                        op0=mybir.AluOpType.mult, op1=mybir.AluOpType.subtract)
                    xhi.append(th); xlo.append(tl)

                for nci, (n0, ns) in enumerate(M_CHUNKS):
                    # qkv matmuls: out [ns, 3C] in 5 o-tiles
                    qkv_sb = qkvsb_p.tile([128, 3 * C], F32, tag="qkvsb")
                    for ot in range(5):
                        o0 = ot * 512
                        osz = min(512, 3 * C - o0)
                        ps = ps_qkv.tile([128, 512], F32, tag="ps_qkv")
                        nmm = 0
                        for c in range(6):
                            for xop in (xhi, xlo):
                                nc.tensor.matmul(
                                    ps[:ns, :osz],
                                    xop[c][:, n0:n0 + ns],
                                    wt[c][:, o0:o0 + osz],
                                    start=(nmm == 0), stop=(nmm == 11))
                                nmm += 1
                        if ot % 2 == 0:
                            nc.vector.tensor_copy(qkv_sb[:ns, o0:o0 + osz], ps[:ns, :osz])
                        else:
                            nc.scalar.copy(qkv_sb[:ns, o0:o0 + osz], ps[:ns, :osz])
                    # l1 over D-segments: [ns, 36]
                    l1 = small_p.tile([128, 36], F32, tag="l1")
                    nc.vector.tensor_reduce(
                        l1[:ns, :], qkv_sb[:ns, :].rearrange("p (s d) -> p s d", d=D),
                        axis=mybir.AxisListType.X, op=mybir.AluOpType.add,
                        apply_absolute_value=True)
                    rho = small_p.tile([128, 36], F32, tag="rho")
                    nc.vector.tensor_scalar(l1[:ns, :], l1[:ns, :], float(S_CONST), None,
                                            op0=mybir.AluOpType.mult)
                    nc.vector.reciprocal(rho[:ns, :], l1[:ns, :])
                    # u*rho + 192 -> bf16 (rounds), per 64-wide segment
                    y192 = y192_p.tile([128, 3 * C], BF16, tag="y192")
                    for s in range(36):
                        eng = nc.vector if s % 2 == 0 else nc.gpsimd
                        eng.tensor_scalar(
                            y192[:ns, s * D:(s + 1) * D], qkv_sb[:ns, s * D:(s + 1) * D],
                            rho[:ns, s:s + 1], M192,
                            op0=mybir.AluOpType.mult, op1=mybir.AluOpType.add)
                    # sign -> ternary; v goes straight into vq store
                    qkq = qkq_p.tile([128, 2 * C], BF16, tag="qkq")
                    nc.scalar.activation(qkq[:ns, :], y192[:ns, :2 * C],
                                         mybir.ActivationFunctionType.Sign, bias=-M192)
                    nc.scalar.activation(vq[b][:ns, nci * C:(nci + 1) * C],
                                         y192[:ns, 2 * C:],
                                         mybir.ActivationFunctionType.Sign, bias=-M192)
                    # transpose q,k blocks to channel-major
                    for j in range(12):
                        pt = ps_tr.tile([128, 128], BF16, tag="ps_tr")
                        nc.tensor.transpose(pt[:, :ns], qkq[:ns, j * 128:(j + 1) * 128],
                                            ident[:ns, :ns])
                        dst = qT[b][j] if j < 6 else kT[b][j - 6]
                        nc.any.tensor_copy(dst[:, n0:n0 + ns], pt[:, :ns])

        # ================= PHASE B: attention =================
        with tc.tile_pool(name="attnsb", bufs=2) as as_p, \
             tc.tile_pool(name="acc", bufs=2) as acc_p, \
             tc.tile_pool(name="stat", bufs=4) as st_p, \
             tc.tile_pool(name="aq", bufs=3) as aq_p, \
             tc.tile_pool(name="outsb", bufs=3) as os_p, \
             tc.tile_pool(name="ps_qk", bufs=3, space="PSUM") as ps_qk, \
             tc.tile_pool(name="ps_av", bufs=3, space="PSUM") as ps_av, \
             tc.tile_pool(name="ps_st", bufs=2, space="PSUM") as ps_st:
            for b in (range(BPC) if _PH in ("full", "B") else []):
                for h in range(H):
                    j, r0 = h // 2, (h % 2) * 64
                    attn_sb = as_p.tile([128, 9 * AS_STRIDE], BF16, tag="attnsb")
                    for mi, (m0, ms) in enumerate(M_CHUNKS):
                        for (n0, nt) in N_TILES:
                            ps = ps_qk.tile([128, 512], F32, tag="ps_qk")
                            nc.tensor.matmul(
                                ps[:ms, :nt],
                                kT[b][j][r0:r0 + 64, m0:m0 + ms],
                                qT[b][j][r0:r0 + 64, n0:n0 + nt],
                                start=True, stop=True)
                            dstap = attn_sb[:ms, mi * AS_STRIDE + n0:mi * AS_STRIDE + n0 + nt]
                            if (mi + (n0 > 0)) % 2 == 0:
                                nc.scalar.copy(dstap, ps[:ms, :nt])
                            else:
                                nc.vector.tensor_copy(dstap, ps[:ms, :nt])
                    # ---- stats: per-chunk |attn| sums split DVE/ACT ----
                    acc = acc_p.tile([128, 16], F32, tag="acc")
                    scr = acc_p.tile([128, N], BF16, tag="scr")
                    for mi in range(8):
                        sl = attn_sb[:, mi * AS_STRIDE:mi * AS_STRIDE + N]
                        if mi % 2 == 0:
                            nc.vector.tensor_reduce(
                                acc[:, mi:mi + 1], sl,
                                axis=mybir.AxisListType.X, op=mybir.AluOpType.add,
                                apply_absolute_value=True)
                        else:
                            nc.scalar.activation(
                                scr[:], sl, mybir.ActivationFunctionType.Abs,
                                accum_out=acc[:, mi:mi + 1])
                    nc.vector.memset(acc[:, 8:9], 0.0)
                    nc.vector.tensor_reduce(
                        acc[0:1, 8:9], attn_sb[0:1, 8 * AS_STRIDE:8 * AS_STRIDE + N],
                        axis=mybir.AxisListType.X, op=mybir.AluOpType.add,
                        apply_absolute_value=True)
                    nc.vector.tensor_reduce(
                        acc[:, 9:10], acc[:, 0:9],
                        axis=mybir.AxisListType.X, op=mybir.AluOpType.add)
                    pstot = ps_st.tile([1, 1], F32, tag="ps_st")
                    nc.tensor.matmul(pstot[:], ones128[:], acc[:, 9:10],
                                     start=True, stop=True)
                    tsc = st_p.tile([1, 2], F32, tag="tsc")
                    nc.vector.tensor_scalar(tsc[0:1, 0:1], pstot[:],
                                            1.0 / (N * N), float(C_EPS),
                                            op0=mybir.AluOpType.mult,
                                            op1=mybir.AluOpType.add)
                    nc.vector.reciprocal(tsc[0:1, 1:2], tsc[0:1, 0:1])
                    rho_b = st_p.tile([128, 2], F32, tag="rho_b")
                    nc.gpsimd.partition_broadcast(rho_b[:, 0:1], tsc[0:1, 1:2])
                    nc.gpsimd.partition_broadcast(rho_b[:, 1:2], tsc[0:1, 0:1])
                    kt_col = st_p.tile([128, 1], F32, tag="kt_col")
                    nc.vector.tensor_scalar(kt_col[:], rho_b[:, 1:2], float(KAPPA), None,
                                            op0=mybir.AluOpType.mult)
                    # ---- quantize chunks + av matmul ----
                    pso = [ps_av.tile([128, 512], F32, tag="ps_av", name=f"pso_{b}_{h}_{i}") for i in range(3)]
                    for mi, (m0, ms) in enumerate(M_CHUNKS):
                        aq = aq_p.tile([128, AS_STRIDE], BF16, tag="aq")
                        nc.vector.tensor_scalar(
                            aq[:ms, :], attn_sb[:ms, mi * AS_STRIDE:(mi + 1) * AS_STRIDE],
                            rho_b[:ms, 0:1], M192,
                            op0=mybir.AluOpType.mult, op1=mybir.AluOpType.add)
                        if mi % 2 == 0:
                            nc.scalar.activation(aq[:ms, :], aq[:ms, :],
                                                 mybir.ActivationFunctionType.Sign,
                                                 bias=-M192)
                        else:
                            nc.vector.tensor_scalar(
                                aq[:ms, :], aq[:ms, :], -M192, -1.0,
                                op0=mybir.AluOpType.add, op1=mybir.AluOpType.max)
                            nc.vector.tensor_scalar(
                                aq[:ms, :], aq[:ms, :], 1.0, None,
                                op0=mybir.AluOpType.min)
                        for ti, (n0, nt) in enumerate(N_TILES):
                            nc.tensor.matmul(
                                pso[ti][:64, :nt],
                                vq[b][:ms, mi * C + h * D:mi * C + h * D + D],
                                aq[:ms, n0:n0 + nt],
                                start=(mi == 0), stop=(mi == 8))
                    for ti, (n0, nt) in enumerate(N_TILES):
                        osb = os_p.tile([64, 512], F32, tag="outsb")
                        nc.vector.tensor_scalar(osb[:, :nt], pso[ti][:64, :nt],
                                                kt_col[0:64, :], None,
                                                op0=mybir.AluOpType.mult)
                        nc.gpsimd.dma_start(
                            y_d[b, h * D:(h + 1) * D, n0:n0 + nt], osb[:, :nt])
    nc.finalize()
    return nc


_NC = None

def _get_nc():
    global _NC
    if _NC is None:
        _NC = build_nc()
    return _NC


def _make_in_maps(x, w_qkv):
    x = np.ascontiguousarray(x, dtype=np.float32)
    w = np.ascontiguousarray(w_qkv, dtype=np.float32)
    s_w = np.float32(np.mean(np.abs(w)) + np.float32(EPS))
    wq_int = np.round(np.clip(w / s_w, -1, 1)).astype(np.float32)  # [3C, C]
    wt = np.ascontiguousarray(wq_int.T).astype(ml_dtypes.bfloat16)  # [C, 3C]
    ident = np.eye(128, dtype=ml_dtypes.bfloat16)
    ones128 = np.ones((128, 1), np.float32)

    in_maps = []
    for core in range(8):
        in_maps.append({
            "x_sh": np.ascontiguousarray(x[core * BPC:(core + 1) * BPC].transpose(0, 2, 1)),
            "wt_bf": wt, "ident": ident, "ones128": ones128,
        })
    return in_maps


def kernel(x, w_qkv):
    in_maps = _make_in_maps(x, w_qkv)
    nc = _get_nc()
    res = run_bass_kernel_spmd(nc, in_maps, core_ids=list(range(8)))
    out = np.empty((B, N, C), np.float32)
    for core in range(8):
        out[core * BPC:(core + 1) * BPC] = res.results[core]["y_sh"].transpose(0, 2, 1)
    return out



# revision 37
# speedup vs baseline: 1.1848x; 1.1848x over previous
"""Trainium2 Bass kernel for ternary-quantized attention (BitNet-style).

Host contract: kernel(x, w_qkv) -> [16,1025,768] fp32.
Shards B=16 over 8 cores (2 batches/core), replicates the ternary weight.

Math (matches fp32 reference to ~0.8% rel err):
  - w ternarized on host to {-1,0,1}; s_w folded out (scale-invariant l1norm).
  - qkv = x @ wt with x as fp32r stationary, wt bf16 moving (exact products,
    fp32 PSUM accumulate).
  - q/k/v quantize: u = t / (l1_row * s_const), s_const = 1/64 + 1e-5;
    ternary = sign(bf16(u + 192) - 192)   [bf16 write rounds to int, RNE]
  - attn_int = q_q @ k_q^T (exact ternary bf16 matmuls, fp32 accum)
  - per-(b,h) scale: t = mean|attn_int| + EPS/(scale*s_const^2); rho = 1/t
  - y = clamp(bf16(attn_int*rho + 192), 191, 193) in {191,192,193}
  - out = (y @ v_q - 192*colsum(v_q)) * (scale * s_const^3 * t)
    [-192*colsum folded in as a rank-1 correction matmul into PSUM]
  - the m=1024 attn row is computed for all 12 heads at once via a
    block-diagonal k-tail stationary matrix (per batch, not per head).
"""
import sys, os
sys.path.insert(0, "/opt/trn_rl_repo")
import numpy as np
import ml_dtypes
from contextlib import ExitStack

import concourse.bass as bass
import concourse.tile as tile
from concourse import bacc
from concourse import mybir
from concourse.bass_utils import run_bass_kernel_spmd

EPS = 1e-5
B, N, C, H, D = 16, 1025, 768, 12, 64
BPC = B // 8  # batches per core
SCALE = float(D) ** -0.5
S_CONST = np.float32(1.0 / D) + np.float32(EPS)
C_EPS = np.float32(EPS) / (np.float32(SCALE) * S_CONST * S_CONST)
KAPPA = np.float32(SCALE) * S_CONST * S_CONST * S_CONST
M192 = 192.0

F32 = mybir.dt.float32
F32R = mybir.dt.float32r
BF16 = mybir.dt.bfloat16

N_CHUNKS = [(i * 128, 128) for i in range(8)] + [(1024, 1)]
M_FULL = [(i * 128, 128) for i in range(8)]  # m-tail row handled batched
AS = 1026  # attn_sb per-m-chunk column stride (even)
QKV_TILES = [(0, 512), (512, 512), (1024, 512), (1536, 512), (2048, 256)]
NT2 = ((0, 512), (512, 512))


def build_nc():
    nc = bacc.Bacc("TRN2", target_bir_lowering=False, debug=False,
                   enable_asserts=False, num_devices=8)
    for val in (-M192, M192):
        t = nc.alloc_sbuf_tensor(f"const-f32-{val}", [128, 1], F32)
        nc.gpsimd.memset(t.ap(), val)
        nc.const_aps.aps[(F32, val)] = t.ap()
    nc.all_engine_barrier()
    xh_d = nc.dram_tensor("x_hi", [BPC, C, N], BF16, kind="ExternalInput").ap()
    xl_d = nc.dram_tensor("x_lo", [BPC, C, N], BF16, kind="ExternalInput").ap()
    wt_d = nc.dram_tensor("wt_bf", [C, 3 * C], BF16, kind="ExternalInput").ap()
    id_d = nc.dram_tensor("ident", [128, 128], BF16, kind="ExternalInput").ap()
    on_d = nc.dram_tensor("ones128", [128, 1], F32, kind="ExternalInput").ap()
    y_d = nc.dram_tensor("y_sh", [BPC, C, N], F32, kind="ExternalOutput").ap()

    with tile.TileContext(nc) as tc, ExitStack() as ctx:
        const_p = ctx.enter_context(tc.tile_pool(name="consts", bufs=1))
        qt_p = ctx.enter_context(tc.tile_pool(name="qt", bufs=6 * BPC))
        kt_p = ctx.enter_context(tc.tile_pool(name="kt", bufs=6 * BPC))
        vq_p = ctx.enter_context(tc.tile_pool(name="vq", bufs=BPC))
        cv_p = ctx.enter_context(tc.tile_pool(name="cv", bufs=BPC))

        ident = const_p.tile([128, 128], BF16, tag="ident")
        nc.sync.dma_start(ident[:], id_d)
        ones128 = const_p.tile([128, 1], F32, tag="ones")
        nc.sync.dma_start(ones128[:], on_d)
        ones128b = const_p.tile([128, 1], BF16, tag="onesb")
        nc.vector.tensor_copy(ones128b[:], ones128[:])
        identf = const_p.tile([128, 128], F32, tag="identf")
        nc.vector.tensor_copy(identf[:], ident[:])

        qT = [[qt_p.tile([128, N], BF16, tag="qt", name=f"qT_{b}_{j}") for j in range(6)] for b in range(BPC)]
        kT = [[kt_p.tile([128, N], BF16, tag="kt", name=f"kT_{b}_{j}") for j in range(6)] for b in range(BPC)]
        vq = [vq_p.tile([128, 9 * C], BF16, tag="vq", name=f"vq_{b}") for b in range(BPC)]
        # colsum(vq) as columns: [128, 6]; col j = c-chunk j (2 heads stacked)
        cvcol = [cv_p.tile([128, 8], F32, tag="cv", name=f"cv_{b}") for b in range(BPC)]

        _PH = os.environ.get("KERNEL_PHASE", "full")
        # ================= PHASE A: qkv + quantize + transpose =================
        with tc.tile_pool(name="wt", bufs=6) as wt_p, \
             tc.tile_pool(name="xs", bufs=6) as xs_p, \
             tc.tile_pool(name="qkvsb", bufs=2) as qkvsb_p, \
             tc.tile_pool(name="small_a", bufs=4) as small_p, \
             tc.tile_pool(name="y192", bufs=2) as y192_p, \
             tc.tile_pool(name="qkq", bufs=2) as qkq_p, \
             tc.tile_pool(name="ps_qkv", bufs=5, space="PSUM") as ps_qkv, \
             tc.tile_pool(name="ps_tr", bufs=3, space="PSUM") as ps_tr:
            wt = []
            for c in range(6):
                t = wt_p.tile([128, 3 * C], BF16, tag="wt")
                nc.sync.dma_start(t[:], wt_d[c * 128:(c + 1) * 128, :])
                wt.append(t)

            pend_tr = []

            def emit_transposes(qkq_t, b, n0, ns):
                for j in range(12):
                    pt = ps_tr.tile([128, 128], BF16, tag="ps_tr")
                    nc.tensor.transpose(pt[:, :ns], qkq_t[:ns, j * 128:(j + 1) * 128],
                                        ident[:ns, :ns])
                    dst = qT[b][j] if j < 6 else kT[b][j - 6]
                    if j % 2 == 0:
                        nc.vector.tensor_copy(dst[:, n0:n0 + ns], pt[:, :ns])
                    else:
                        nc.scalar.copy(dst[:, n0:n0 + ns], pt[:, :ns])

            for b in (range(BPC) if _PH in ("full", "A") else []):
                xs = []
                for c in range(6):
                    xh = xs_p.tile([128, N], BF16, tag="xh")
                    nc.sync.dma_start(xh[:], xh_d[b, c * 128:(c + 1) * 128, :])
                    xl = xs_p.tile([128, N], BF16, tag="xl")
                    nc.sync.dma_start(xl[:], xl_d[b, c * 128:(c + 1) * 128, :])
                    xs.append((xh, xl))

                for nci, (n0, ns) in enumerate(N_CHUNKS):
                    qkv_sb = qkvsb_p.tile([128, 3 * C], F32, tag="qkvsb")
                    pss = []
                    for ti, (o0, osz) in enumerate(QKV_TILES):
                        ps = ps_qkv.tile([128, 512], F32, tag="ps_qkv")
                        nmm = 0
                        for c in range(6):
                            for xop in xs[c]:
                                nc.tensor.matmul(
                                    ps[:ns, :osz],
                                    xop[:, n0:n0 + ns],
                                    wt[c][:, o0:o0 + osz],
                                    start=(nmm == 0), stop=(nmm == 11))
                                nmm += 1
                        pss.append((ps, o0, osz))
                    for ti, (ps, o0, osz) in enumerate(pss):
                        if ti in (0, 2):
                            nc.vector.tensor_copy(qkv_sb[:ns, o0:o0 + osz], ps[:ns, :osz])
                        else:
                            nc.scalar.copy(qkv_sb[:ns, o0:o0 + osz], ps[:ns, :osz])
                    # l1 over D-segments: [ns, 36] (DVE; gpsimd can't free-reduce)
                    l1 = small_p.tile([128, 36], F32, tag="l1")
                    nc.vector.tensor_reduce(
                        l1[:ns, :], qkv_sb[:ns, :].rearrange("p (s d) -> p s d", d=D),
                        axis=mybir.AxisListType.X, op=mybir.AluOpType.add,
                        apply_absolute_value=True)
                    rho = small_p.tile([128, 36], F32, tag="rho")
                    nc.vector.tensor_scalar(l1[:ns, :], l1[:ns, :], float(S_CONST), None,
                                            op0=mybir.AluOpType.mult)
                    nc.vector.reciprocal(rho[:ns, :], l1[:ns, :])
                    # u*rho + 192 -> bf16 (rounds); 36 segs split DVE/Pool/ACT
                    y192 = y192_p.tile([128, 3 * C], BF16, tag="y192")
                    for s in range(36):
                        sl_in = qkv_sb[:ns, s * D:(s + 1) * D]
                        sl_out = y192[:ns, s * D:(s + 1) * D]
                        r = s % 3
                        if r == 0:
                            nc.vector.tensor_scalar(
                                sl_out, sl_in, rho[:ns, s:s + 1], M192,
                                op0=mybir.AluOpType.mult, op1=mybir.AluOpType.add)
                        elif r == 1:
                            nc.gpsimd.tensor_scalar(
                                sl_out, sl_in, rho[:ns, s:s + 1], M192,
                                op0=mybir.AluOpType.mult, op1=mybir.AluOpType.add)
                        else:
                            nc.scalar.activation(
                                sl_out, sl_in, mybir.ActivationFunctionType.Identity,
                                bias=M192, scale=rho[:ns, s:s + 1])
                    # q,k ternary via ACT Sign -> qkq
                    qkq = qkq_p.tile([128, 2 * C], BF16, tag="qkq")
                    nc.scalar.activation(qkq[:ns, :], y192[:ns, :2 * C],
                                         mybir.ActivationFunctionType.Sign, bias=-M192)
                    # v ternary via DVE clamp + sub -> vq
                    nc.vector.tensor_scalar(
                        y192[:ns, 2 * C:], y192[:ns, 2 * C:], 193.0, 191.0,
                        op0=mybir.AluOpType.min, op1=mybir.AluOpType.max)
                    nc.vector.tensor_scalar(
                        vq[b][:ns, nci * C:(nci + 1) * C], y192[:ns, 2 * C:],
                        M192, None, op0=mybir.AluOpType.subtract)
                    if pend_tr:
                        emit_transposes(*pend_tr.pop())
                    pend_tr.append((qkq, b, n0, ns))
                if pend_tr:
                    emit_transposes(*pend_tr.pop())

        # ================= PHASE B: attention =================
        with tc.tile_pool(name="attnsb", bufs=2) as as_p, \
             tc.tile_pool(name="ytile", bufs=2) as y_p, \
             tc.tile_pool(name="mt", bufs=2) as mt_p, \
             tc.tile_pool(name="acc", bufs=2) as acc_p, \
             tc.tile_pool(name="stat", bufs=4) as st_p, \
             tc.tile_pool(name="outsb", bufs=3) as os_p, \
             tc.tile_pool(name="ps_qk", bufs=2, space="PSUM") as ps_qk, \
             tc.tile_pool(name="ps_av", bufs=3, space="PSUM") as ps_av, \
             tc.tile_pool(name="ps_tl", bufs=1, space="PSUM") as ps_tl:
            def emit_batch_prep(b):
                """colsum(vq), batched m=1024 attn rows for all heads, vtail bcast."""
                # --- colsum of vq, column-oriented: out [128, 6], col j = c-chunk ---
                cvps = ps_tl.tile([128, 16], F32, tag="ps_tl", name=f"cvps_{b}")
                for cc in range(6):
                    for mi in range(9):
                        ms = 128 if mi < 8 else 1
                        nc.tensor.matmul(
                            cvps[:, cc:cc + 1],
                            vq[b][:ms, mi * C + cc * 128:mi * C + (cc + 1) * 128],
                            ones128b[:ms, :],
                            start=(mi == 0), stop=(mi == 8))
                nc.vector.tensor_copy(cvcol[b][:, 0:6], cvps[:, 0:6])
                # --- batched m-tail: attn[1024, n] for all 12 heads ---
                ktd = mt_p.tile([128, 72], BF16, tag="ktd", name=f"ktd_{b}")
                nc.vector.memset(ktd[:], 0.0)
                for h in range(H):
                    j, r0 = h // 2, (h % 2) * 64
                    nc.vector.tensor_copy(ktd[r0:r0 + 64, j * 12 + h:j * 12 + h + 1],
                                          kT[b][j][r0:r0 + 64, 1024:1025])
                mtps = ps_qk.tile([128, 1024], F32, tag="ps_qk", name=f"mtps_{b}")
                mtt = ps_tl.tile([128, 16], F32, tag="ps_tl", name=f"mtt_{b}")
                for j in range(6):
                    for (n0, nt) in NT2:
                        nc.tensor.matmul(
                            mtps[0:12, n0:n0 + nt],
                            ktd[:, j * 12:(j + 1) * 12],
                            qT[b][j][:, n0:n0 + nt],
                            start=(j == 0), stop=(j == 5))
                    # attn[1024,1024] for all heads -> [12, 1]
                    nc.tensor.matmul(
                        mtt[0:12, 0:1],
                        ktd[:, j * 12:(j + 1) * 12],
                        qT[b][j][:, 1024:1025],
                        start=(j == 0), stop=(j == 5))
                mt_sb = mt_p.tile([12, AS], BF16, tag="mt", name=f"mt_{b}")
                nc.scalar.copy(mt_sb[:, 0:1024], mtps[0:12, 0:1024])
                nc.vector.tensor_copy(mt_sb[:, 1024:1025], mtt[0:12, 0:1])
                nc.vector.memset(mt_sb[:, 1025:1026], 0.0)
                # per-head |.| partials over the m-tail row: [12, 1]
                mta = mt_p.tile([12, 1], F32, tag="mta", name=f"mta_{b}")
                nc.vector.tensor_reduce(
                    mta[:], mt_sb[:, 0:1025],
                    axis=mybir.AxisListType.X, op=mybir.AluOpType.add,
                    apply_absolute_value=True)
                # v tail row broadcast to partitions 0..11 for per-head AV
                vtb = mt_p.tile([12, C], BF16, tag="vtb", name=f"vtb_{b}")
                nc.gpsimd.partition_broadcast(vtb[:], vq[b][0:1, 8 * C:9 * C])
                return mt_sb, mta, vtb

            def emit_qk(b, h, ptail):
                j, r0 = h // 2, (h % 2) * 64
                qkt = []
                for mi, (m0, ms) in enumerate(M_FULL):
                    ps = ps_qk.tile([128, 1024], F32, tag="ps_qk")
                    for (n0, nt) in NT2:
                        nc.tensor.matmul(
                            ps[:ms, n0:n0 + nt],
                            kT[b][j][r0:r0 + 64, m0:m0 + ms],
                            qT[b][j][r0:r0 + 64, n0:n0 + nt],
                            start=True, stop=True)
                    nc.tensor.matmul(
                        ptail[:ms, mi:mi + 1],
                        kT[b][j][r0:r0 + 64, m0:m0 + ms],
                        qT[b][j][r0:r0 + 64, 1024:1025],
                        start=True, stop=True)
                    qkt.append((ps, mi, ms))
                return qkt

            def emit_drain_stats(b, h, attn_sb, yt, qkt, ptail, acc, tsc, rho_b, mta):
                for (ps, mi, ms) in qkt:
                    dst = attn_sb[:ms, mi * AS:mi * AS + 1024]
                    if mi in (0, 2, 4):
                        nc.vector.tensor_copy(dst, ps[:ms, :1024])
                    else:
                        nc.scalar.copy(dst, ps[:ms, :1024])
                chunks3d = attn_sb[:].rearrange("p (s d) -> p s d", d=AS)
                nc.vector.tensor_copy(
                    chunks3d[:, :, 1024:1025],
                    ptail[:, 0:8].rearrange("p (s d) -> p s d", d=1))
                nc.gpsimd.memset(chunks3d[:, :, 1025:1026], 0.0)
                # stats: DVE chunks 0-2, ACT chunks 3-7 (abs+accum), mtail partial
                nc.vector.tensor_reduce(
                    acc[:, 0:3], attn_sb[:, 0:3 * AS].rearrange("p (s d) -> p s d", d=AS),
                    axis=mybir.AxisListType.X, op=mybir.AluOpType.add,
                    apply_absolute_value=True)
                # abs scratch lands in yt (quant overwrites it afterwards)
                nc.scalar.activation(
                    yt[:, 3 * AS:8 * AS], attn_sb[:, 3 * AS:8 * AS],
                    mybir.ActivationFunctionType.Abs, accum_out=acc[:, 3:4])
                nc.vector.tensor_reduce(
                    acc[:, 4:5], acc[:, 0:4],
                    axis=mybir.AxisListType.X, op=mybir.AluOpType.add)
                pstot = ptail[0:1, 12:13]
                nc.tensor.matmul(pstot, ones128[:], acc[:, 4:5], start=True, stop=False)
                # += |m-tail row| partial of this head (one-hot select via identf)
                nc.tensor.matmul(pstot, mta[:], identf[0:12, h:h + 1],
                                 start=False, stop=True)
                nc.vector.tensor_scalar(tsc[0:1, 0:1], pstot,
                                        1.0 / (N * N), float(C_EPS),
                                        op0=mybir.AluOpType.mult,
                                        op1=mybir.AluOpType.add)
                nc.vector.reciprocal(tsc[0:1, 1:2], tsc[0:1, 0:1])
                nc.gpsimd.partition_broadcast(rho_b[:, 0:1], tsc[0:1, 1:2])
                nc.gpsimd.partition_broadcast(rho_b[:, 1:2], tsc[0:1, 0:1])
                nc.vector.tensor_scalar(rho_b[:, 2:3], rho_b[:, 1:2], float(KAPPA), None,
                                        op0=mybir.AluOpType.mult)
                # -192*colsum(v) bias for this head, partition-aligned to 0..63
                cvs = cvcol[b][(h % 2) * 64:(h % 2) * 64 + 64, h // 2:h // 2 + 1]
                nc.vector.tensor_scalar(rho_b[0:64, 3:4], cvs, -M192, None,
                                        op0=mybir.AluOpType.mult)
                # bias*kt for the ACT osb path
                nc.vector.tensor_scalar(rho_b[0:64, 4:5], rho_b[0:64, 3:4],
                                        rho_b[0:64, 2:3], None,
                                        op0=mybir.AluOpType.mult)

            def emit_quant(attn_sb, yt, ytail, mt_sb, rho_b):
                # y = clamp(bf16(a*rho + 192), 191, 193); groups (0-3), (4-7)
                for g in range(2):
                    sl_in = attn_sb[:, g * 4 * AS:(g + 1) * 4 * AS]
                    sl_out = yt[:, g * 4 * AS:(g + 1) * 4 * AS]
                    nc.vector.tensor_scalar(
                        sl_out, sl_in, rho_b[:, 0:1], M192,
                        op0=mybir.AluOpType.mult, op1=mybir.AluOpType.add)
                    if g == 0:
                        nc.vector.tensor_scalar(
                            sl_out, sl_out, 193.0, 191.0,
                            op0=mybir.AluOpType.min, op1=mybir.AluOpType.max)
                    else:
                        nc.gpsimd.tensor_scalar(
                            sl_out, sl_out, 193.0, 191.0,
                            op0=mybir.AluOpType.min, op1=mybir.AluOpType.max)
                # m-tail row for this head
                nc.vector.tensor_scalar(
                    ytail[:], mt_sb[:], rho_b[0:12, 0:1], M192,
                    op0=mybir.AluOpType.mult, op1=mybir.AluOpType.add)
                nc.vector.tensor_scalar(
                    ytail[:], ytail[:], 193.0, 191.0,
                    op0=mybir.AluOpType.min, op1=mybir.AluOpType.max)

            def emit_av(b, h, yt, ytail, vtb, rho_b):
                pso = [ps_av.tile([128, 512], F32, tag="ps_av",
                                  name=f"pso_{b}_{h}_{i}") for i in range(2)]
                pst = ps_av.tile([128, 16], F32, tag="ps_av", name=f"psot_{b}_{h}")
                for mi, (m0, ms) in enumerate(M_FULL):
                    vsl = vq[b][:ms, mi * C + h * D:mi * C + h * D + D]
                    for ti, (n0, nt) in enumerate(NT2):
                        nc.tensor.matmul(
                            pso[ti][:64, :nt], vsl, yt[:ms, mi * AS + n0:mi * AS + n0 + nt],
                            start=(mi == 0), stop=False)
                    nc.tensor.matmul(
                        pst[:64, 0:1], vsl, yt[:ms, mi * AS + 1024:mi * AS + 1025],
                        start=(mi == 0), stop=False)
                # m-tail row contribution: one-hot-masked v-tail, 12-partition contract
                vmask = mt_p.tile([12, D], BF16, tag="vmask")
                nc.vector.tensor_scalar(vmask[:], vtb[0:12, h * D:(h + 1) * D],
                                        identf[0:12, h:h + 1], None,
                                        op0=mybir.AluOpType.mult)
                for ti, (n0, nt) in enumerate(NT2):
                    nc.tensor.matmul(pso[ti][:64, :nt], vmask[:], ytail[0:12, n0:n0 + nt],
                                     start=False, stop=True)
                nc.tensor.matmul(pst[:64, 0:1], vmask[:], ytail[0:12, 1024:1025],
                                 start=False, stop=True)
                return pso, pst

            def emit_out(b, h, pso, pst, rho_b):
                # out = (pso - 192*colsum_v) * kt
                kt_col = rho_b[0:64, 2:3]
                bias_col = rho_b[0:64, 3:4]
                biaskt_col = rho_b[0:64, 4:5]
                for ti, (n0, nt) in enumerate(NT2):
                    osb = os_p.tile([64, 512], F32, tag="outsb")
                    if ti == 0:
                        nc.vector.tensor_scalar(osb[:, :nt], pso[ti][:64, :nt],
                                                bias_col, kt_col,
                                                op0=mybir.AluOpType.add,
                                                op1=mybir.AluOpType.mult)
                    else:
                        nc.scalar.activation(osb[:, :nt], pso[ti][:64, :nt],
                                             mybir.ActivationFunctionType.Identity,
                                             scale=kt_col, bias=biaskt_col)
                    nc.sync.dma_start(y_d[b, h * D:(h + 1) * D, n0:n0 + nt], osb[:, :nt])
                osb = os_p.tile([64, 512], F32, tag="outsb")
                nc.vector.tensor_scalar(osb[:, 0:1], pst[:64, 0:1],
                                        bias_col, kt_col,
                                        op0=mybir.AluOpType.add,
                                        op1=mybir.AluOpType.mult)
                nc.sync.dma_start(y_d[b, h * D:(h + 1) * D, 1024:1025], osb[:, 0:1])

            for b in (range(BPC) if _PH in ("full", "B") else []):
                mt_sb, mta, vtb = emit_batch_prep(b)
                prev = None
                for h in range(H):
                    attn_sb = as_p.tile([128, 8 * AS], BF16, tag="attnsb")
                    yt = y_p.tile([128, 8 * AS], BF16, tag="ytile")
                    ytail = mt_p.tile([12, AS], BF16, tag="ytail")
                    acc = acc_p.tile([128, 6], F32, tag="acc")
                    tsc = st_p.tile([1, 2], F32, tag="tsc")
                    rho_b = st_p.tile([128, 5], F32, tag="rho_b")
                    ptail = ps_tl.tile([128, 16], F32, tag="ps_tl")
                    qkt = emit_qk(b, h, ptail)
                    if prev is not None:
                        pv = prev
                        pso, pst = emit_av(b, pv["h"], pv["yt"], pv["ytail"], vtb, pv["rho_b"])
                        emit_out(b, pv["h"], pso, pst, pv["rho_b"])
                    emit_drain_stats(b, h, attn_sb, yt, qkt, ptail, acc, tsc, rho_b, mta)
                    emit_quant(attn_sb, yt, ytail, mt_sb, rho_b)
                    prev = {"h": h, "yt": yt, "ytail": ytail, "rho_b": rho_b}
                pv = prev
                pso, pst = emit_av(b, pv["h"], pv["yt"], pv["ytail"], vtb, pv["rho_b"])
                emit_out(b, pv["h"], pso, pst, pv["rho_b"])
    nc.finalize()
    return nc


_NC = None

def _get_nc():
    global _NC
    if _NC is None:
        _NC = build_nc()
    return _NC


def _make_in_maps(x, w_qkv):
    x = np.ascontiguousarray(x, dtype=np.float32)
    w = np.ascontiguousarray(w_qkv, dtype=np.float32)
    s_w = np.float32(np.mean(np.abs(w)) + np.float32(EPS))
    wq_int = np.round(np.clip(w / s_w, -1, 1)).astype(np.float32)  # [3C, C]
    wt = np.ascontiguousarray(wq_int.T).astype(ml_dtypes.bfloat16)  # [C, 3C]
    ident = np.eye(128, dtype=ml_dtypes.bfloat16)
    ones128 = np.ones((128, 1), np.float32)

    xt = x.transpose(0, 2, 1)  # [B, C, N]
    xh = xt.astype(ml_dtypes.bfloat16)
    xl = (xt - xh.astype(np.float32)).astype(ml_dtypes.bfloat16)

    in_maps = []
    for core in range(8):
        sl = slice(core * BPC, (core + 1) * BPC)
        in_maps.append({
            "x_hi": np.ascontiguousarray(xh[sl]),
            "x_lo": np.ascontiguousarray(xl[sl]),
            "wt_bf": wt, "ident": ident, "ones128": ones128,
        })
    return in_maps


def kernel(x, w_qkv):
    in_maps = _make_in_maps(x, w_qkv)
    nc = _get_nc()
    res = run_bass_kernel_spmd(nc, in_maps, core_ids=list(range(8)))
    out = np.empty((B, N, C), np.float32)
    for core in range(8):
        out[core * BPC:(core + 1) * BPC] = res.results[core]["y_sh"].transpose(0, 2, 1)
    return out


# revision 47
# speedup vs baseline: 1.3151x; 1.1100x over previous
"""Trainium2 Bass kernel for ternary-quantized attention (BitNet-style).

Host contract: kernel(x, w_qkv) -> [16,1025,768] fp32.
Shards B=16 over 8 cores (2 batches/core), replicates the ternary weight.

Math (matches fp32 reference to ~0.8% rel err):
  - w ternarized on host to {-1,0,1}; s_w folded out (scale-invariant l1norm).
  - qkv = x @ wt with x as fp32r stationary, wt bf16 moving (exact products,
    fp32 PSUM accumulate).
  - q/k/v quantize: u = t / (l1_row * s_const), s_const = 1/64 + 1e-5;
    ternary = sign(bf16(u + 192) - 192)   [bf16 write rounds to int, RNE]
  - attn_int = q_q @ k_q^T (exact ternary bf16 matmuls, fp32 accum)
  - per-(b,h) scale: t = mean|attn_int| + EPS/(scale*s_const^2); rho = 1/t
  - y = clamp(bf16(attn_int*rho + 192), 191, 193) in {191,192,193}
  - out = (y @ v_q - 192*colsum(v_q)) * (scale * s_const^3 * t)
    [-192*colsum folded in as a rank-1 correction matmul into PSUM]
  - the m=1024 attn row is computed for all 12 heads at once via a
    block-diagonal k-tail stationary matrix (per batch, not per head).
"""
import sys, os
sys.path.insert(0, "/opt/trn_rl_repo")
import numpy as np
import ml_dtypes
from contextlib import ExitStack

import concourse.bass as bass
import concourse.tile as tile
from concourse import bacc
from concourse import mybir
from concourse import bass_isa
from concourse.bass_utils import run_bass_kernel_spmd

EPS = 1e-5
B, N, C, H, D = 16, 1025, 768, 12, 64
BPC = B // 8  # batches per core
SCALE = float(D) ** -0.5
S_CONST = np.float32(1.0 / D) + np.float32(EPS)
C_EPS = np.float32(EPS) / (np.float32(SCALE) * S_CONST * S_CONST)
KAPPA = np.float32(SCALE) * S_CONST * S_CONST * S_CONST
M192 = 192.0

F32 = mybir.dt.float32
F32R = mybir.dt.float32r
BF16 = mybir.dt.bfloat16

N_CHUNKS = [(i * 128, 128) for i in range(8)] + [(1024, 1)]
M_FULL = [(i * 128, 128) for i in range(8)]  # m-tail row handled batched
AS = 1026  # attn_sb per-m-chunk column stride (even)
QKV_TILES = [(0, 512), (512, 512), (1024, 512), (1536, 512), (2048, 256)]
NT2 = ((0, 512), (512, 512))


def build_nc():
    nc = bacc.Bacc("TRN2", target_bir_lowering=False, debug=False,
                   enable_asserts=False, num_devices=8)
    for val in (-M192, M192):
        t = nc.alloc_sbuf_tensor(f"const-f32-{val}", [128, 1], F32)
        nc.gpsimd.memset(t.ap(), val)
        nc.const_aps.aps[(F32, val)] = t.ap()
    nc.all_engine_barrier()
    xh_d = nc.dram_tensor("x_hi", [BPC, C, N], BF16, kind="ExternalInput").ap()
    xl_d = nc.dram_tensor("x_lo", [BPC, C, N], BF16, kind="ExternalInput").ap()
    wt_d = nc.dram_tensor("wt_bf", [C, 3 * C], BF16, kind="ExternalInput").ap()
    id_d = nc.dram_tensor("ident", [128, 128], BF16, kind="ExternalInput").ap()
    on_d = nc.dram_tensor("ones128", [128, 1], F32, kind="ExternalInput").ap()
    y_d = nc.dram_tensor("y_sh", [BPC, C, N], F32, kind="ExternalOutput").ap()

    with tile.TileContext(nc) as tc, ExitStack() as ctx:
        const_p = ctx.enter_context(tc.tile_pool(name="consts", bufs=1))
        qt_p = ctx.enter_context(tc.tile_pool(name="qt", bufs=6 * BPC))
        kt_p = ctx.enter_context(tc.tile_pool(name="kt", bufs=6 * BPC))
        vq_p = ctx.enter_context(tc.tile_pool(name="vq", bufs=BPC))
        cv_p = ctx.enter_context(tc.tile_pool(name="cv", bufs=BPC))

        ident = const_p.tile([128, 128], BF16, tag="ident")
        nc.sync.dma_start(ident[:], id_d)
        ones128 = const_p.tile([128, 1], F32, tag="ones")
        nc.sync.dma_start(ones128[:], on_d)
        ones128b = const_p.tile([128, 1], BF16, tag="onesb")
        nc.vector.tensor_copy(ones128b[:], ones128[:])
        identf = const_p.tile([128, 128], F32, tag="identf")
        nc.vector.tensor_copy(identf[:], ident[:])

        qT = [[qt_p.tile([128, N], BF16, tag="qt", name=f"qT_{b}_{j}") for j in range(6)] for b in range(BPC)]
        kT = [[kt_p.tile([128, N], BF16, tag="kt", name=f"kT_{b}_{j}") for j in range(6)] for b in range(BPC)]
        vq = [vq_p.tile([128, 9 * C], BF16, tag="vq", name=f"vq_{b}") for b in range(BPC)]
        # colsum(vq) as columns: [128, 6]; col j = c-chunk j (2 heads stacked)
        cvcol = [cv_p.tile([128, 8], F32, tag="cv", name=f"cv_{b}") for b in range(BPC)]

        _PH = os.environ.get("KERNEL_PHASE", "full")
        # ================= PHASE A: qkv + quantize + transpose =================
        with tc.tile_pool(name="wt", bufs=6) as wt_p, \
             tc.tile_pool(name="xs", bufs=6) as xs_p, \
             tc.tile_pool(name="qkvsb", bufs=2) as qkvsb_p, \
             tc.tile_pool(name="small_a", bufs=4) as small_p, \
             tc.tile_pool(name="y192", bufs=2) as y192_p, \
             tc.tile_pool(name="qkq", bufs=2) as qkq_p, \
             tc.tile_pool(name="ps_qkv", bufs=5, space="PSUM") as ps_qkv, \
             tc.tile_pool(name="ps_tr", bufs=3, space="PSUM") as ps_tr:
            wt = []
            for c in range(6):
                t = wt_p.tile([128, 3 * C], BF16, tag="wt")
                nc.sync.dma_start(t[:], wt_d[c * 128:(c + 1) * 128, :])
                wt.append(t)

            pend_tr = []

            def emit_transposes(qkq_t, b, n0, ns):
                for j in range(12):
                    pt = ps_tr.tile([128, 128], BF16, tag="ps_tr")
                    nc.tensor.transpose(pt[:, :ns], qkq_t[:ns, j * 128:(j + 1) * 128],
                                        ident[:ns, :ns])
                    dst = qT[b][j] if j < 6 else kT[b][j - 6]
                    if j % 2 == 0:
                        nc.vector.tensor_copy(dst[:, n0:n0 + ns], pt[:, :ns])
                    else:
                        nc.scalar.copy(dst[:, n0:n0 + ns], pt[:, :ns])

            for b in (range(BPC) if _PH in ("full", "A") else []):
                xs = []
                for c in range(6):
                    xh = xs_p.tile([128, N], BF16, tag="xh")
                    nc.sync.dma_start(xh[:], xh_d[b, c * 128:(c + 1) * 128, :])
                    xl = xs_p.tile([128, N], BF16, tag="xl")
                    nc.sync.dma_start(xl[:], xl_d[b, c * 128:(c + 1) * 128, :])
                    xs.append((xh, xl))

                for nci, (n0, ns) in enumerate(N_CHUNKS):
                    qkv_sb = qkvsb_p.tile([128, 3 * C], F32, tag="qkvsb")
                    pss = []
                    for ti, (o0, osz) in enumerate(QKV_TILES):
                        ps = ps_qkv.tile([128, 512], F32, tag="ps_qkv")
                        nmm = 0
                        for c in range(6):
                            for xop in xs[c]:
                                nc.tensor.matmul(
                                    ps[:ns, :osz],
                                    xop[:, n0:n0 + ns],
                                    wt[c][:, o0:o0 + osz],
                                    start=(nmm == 0), stop=(nmm == 11))
                                nmm += 1
                        pss.append((ps, o0, osz))
                    for ti, (ps, o0, osz) in enumerate(pss):
                        if ti in (0, 2):
                            nc.vector.tensor_copy(qkv_sb[:ns, o0:o0 + osz], ps[:ns, :osz])
                        else:
                            nc.scalar.copy(qkv_sb[:ns, o0:o0 + osz], ps[:ns, :osz])
                    # l1 over D-segments: [ns, 36] (DVE; gpsimd can't free-reduce)
                    l1 = small_p.tile([128, 36], F32, tag="l1")
                    nc.vector.tensor_reduce(
                        l1[:ns, :], qkv_sb[:ns, :].rearrange("p (s d) -> p s d", d=D),
                        axis=mybir.AxisListType.X, op=mybir.AluOpType.add,
                        apply_absolute_value=True)
                    rho = small_p.tile([128, 36], F32, tag="rho")
                    nc.vector.tensor_scalar(l1[:ns, :], l1[:ns, :], float(S_CONST), None,
                                            op0=mybir.AluOpType.mult)
                    nc.vector.reciprocal(rho[:ns, :], l1[:ns, :])
                    # u*rho + 192 -> bf16 (rounds); 36 segs split DVE/Pool/ACT
                    y192 = y192_p.tile([128, 3 * C], BF16, tag="y192")
                    for s in range(36):
                        sl_in = qkv_sb[:ns, s * D:(s + 1) * D]
                        sl_out = y192[:ns, s * D:(s + 1) * D]
                        r = s % 3
                        if r == 0:
                            nc.vector.tensor_scalar(
                                sl_out, sl_in, rho[:ns, s:s + 1], M192,
                                op0=mybir.AluOpType.mult, op1=mybir.AluOpType.add)
                        elif r == 1:
                            nc.gpsimd.tensor_scalar(
                                sl_out, sl_in, rho[:ns, s:s + 1], M192,
                                op0=mybir.AluOpType.mult, op1=mybir.AluOpType.add)
                        else:
                            nc.scalar.activation(
                                sl_out, sl_in, mybir.ActivationFunctionType.Identity,
                                bias=M192, scale=rho[:ns, s:s + 1])
                    # q,k ternary via ACT Sign -> qkq
                    qkq = qkq_p.tile([128, 2 * C], BF16, tag="qkq")
                    nc.scalar.activation(qkq[:ns, :], y192[:ns, :2 * C],
                                         mybir.ActivationFunctionType.Sign, bias=-M192)
                    # v ternary via DVE clamp + sub -> vq
                    nc.vector.tensor_scalar(
                        y192[:ns, 2 * C:], y192[:ns, 2 * C:], 193.0, 191.0,
                        op0=mybir.AluOpType.min, op1=mybir.AluOpType.max)
                    nc.vector.tensor_scalar(
                        vq[b][:ns, nci * C:(nci + 1) * C], y192[:ns, 2 * C:],
                        M192, None, op0=mybir.AluOpType.subtract)
                    if pend_tr:
                        emit_transposes(*pend_tr.pop())
                    pend_tr.append((qkq, b, n0, ns))
                if pend_tr:
                    emit_transposes(*pend_tr.pop())

        # ================= PHASE B: attention =================
        with tc.tile_pool(name="attnsb", bufs=2) as as_p, \
             tc.tile_pool(name="ytile", bufs=2) as y_p, \
             tc.tile_pool(name="mt", bufs=2) as mt_p, \
             tc.tile_pool(name="acc", bufs=2) as acc_p, \
             tc.tile_pool(name="stat", bufs=4) as st_p, \
             tc.tile_pool(name="outsb", bufs=3) as os_p, \
             tc.tile_pool(name="ps_qk", bufs=2, space="PSUM") as ps_qk, \
             tc.tile_pool(name="ps_av", bufs=3, space="PSUM") as ps_av, \
             tc.tile_pool(name="ps_tl", bufs=1, space="PSUM") as ps_tl:
            def emit_batch_prep(b):
                """colsum(vq), batched m=1024 attn rows for all heads, vtail bcast."""
                # --- colsum of vq, column-oriented: out [128, 6], col j = c-chunk ---
                cvps = ps_tl.tile([128, 16], F32, tag="ps_tl", name=f"cvps_{b}")
                for cc in range(6):
                    for mi in range(9):
                        ms = 128 if mi < 8 else 1
                        nc.tensor.matmul(
                            cvps[:, cc:cc + 1],
                            vq[b][:ms, mi * C + cc * 128:mi * C + (cc + 1) * 128],
                            ones128b[:ms, :],
                            start=(mi == 0), stop=(mi == 8))
                nc.vector.tensor_copy(cvcol[b][:, 0:6], cvps[:, 0:6])
                # --- batched m-tail: attn[1024, n] for all 12 heads ---
                ktd = mt_p.tile([128, 72], BF16, tag="ktd", name=f"ktd_{b}")
                nc.vector.memset(ktd[:], 0.0)
                for h in range(H):
                    j, r0 = h // 2, (h % 2) * 64
                    nc.vector.tensor_copy(ktd[r0:r0 + 64, j * 12 + h:j * 12 + h + 1],
                                          kT[b][j][r0:r0 + 64, 1024:1025])
                mtps = ps_qk.tile([128, 1024], F32, tag="ps_qk", name=f"mtps_{b}")
                mtt = ps_tl.tile([128, 16], F32, tag="ps_tl", name=f"mtt_{b}")
                for j in range(6):
                    for (n0, nt) in NT2:
                        nc.tensor.matmul(
                            mtps[0:12, n0:n0 + nt],
                            ktd[:, j * 12:(j + 1) * 12],
                            qT[b][j][:, n0:n0 + nt],
                            start=(j == 0), stop=(j == 5))
                    # attn[1024,1024] for all heads -> [12, 1]
                    nc.tensor.matmul(
                        mtt[0:12, 0:1],
                        ktd[:, j * 12:(j + 1) * 12],
                        qT[b][j][:, 1024:1025],
                        start=(j == 0), stop=(j == 5))
                mt_sb = mt_p.tile([12, AS], BF16, tag="mt", name=f"mt_{b}")
                nc.scalar.copy(mt_sb[:, 0:1024], mtps[0:12, 0:1024])
                nc.vector.tensor_copy(mt_sb[:, 1024:1025], mtt[0:12, 0:1])
                nc.vector.memset(mt_sb[:, 1025:1026], 0.0)
                # per-head |.| partials over the m-tail row: [12, 1]
                mta = mt_p.tile([12, 1], F32, tag="mta", name=f"mta_{b}")
                nc.vector.tensor_reduce(
                    mta[:], mt_sb[:, 0:1025],
                    axis=mybir.AxisListType.X, op=mybir.AluOpType.add,
                    apply_absolute_value=True)
                # transpose to a [1, 12] row so per-head scalars are partition-0
                nc.tensor.matmul(mtt[0:1, 2:14], mta[:], identf[0:12, 0:12],
                                 start=True, stop=True)
                mta_row = mt_p.tile([1, 12], F32, tag="mtarow", name=f"mtarow_{b}")
                nc.vector.tensor_copy(mta_row[:], mtt[0:1, 2:14])
                # v tail row broadcast to partitions 0..11 for per-head AV
                vtb = mt_p.tile([12, C], BF16, tag="vtb", name=f"vtb_{b}")
                nc.gpsimd.partition_broadcast(vtb[:], vq[b][0:1, 8 * C:9 * C])
                return mt_sb, mta_row, vtb

            def emit_qk(b, h, ptail):
                j, r0 = h // 2, (h % 2) * 64
                qkt = []
                for mi, (m0, ms) in enumerate(M_FULL):
                    ps = ps_qk.tile([128, 1024], F32, tag="ps_qk")
                    for (n0, nt) in NT2:
                        nc.tensor.matmul(
                            ps[:ms, n0:n0 + nt],
                            kT[b][j][r0:r0 + 64, m0:m0 + ms],
                            qT[b][j][r0:r0 + 64, n0:n0 + nt],
                            start=True, stop=True)
                    nc.tensor.matmul(
                        ptail[:ms, mi:mi + 1],
                        kT[b][j][r0:r0 + 64, m0:m0 + ms],
                        qT[b][j][r0:r0 + 64, 1024:1025],
                        start=True, stop=True)
                    qkt.append((ps, mi, ms))
                return qkt

            def emit_drain_stats(b, h, attn_sb, yt, qkt, ptail, acc, rho_b, mta_row):
                for (ps, mi, ms) in qkt:
                    dst = attn_sb[:ms, mi * AS:mi * AS + 1024]
                    if mi in (0, 2):
                        nc.vector.tensor_copy(dst, ps[:ms, :1024])
                    else:
                        nc.scalar.copy(dst, ps[:ms, :1024])
                chunks3d = attn_sb[:].rearrange("p (s d) -> p s d", d=AS)
                nc.vector.tensor_copy(
                    chunks3d[:, :, 1024:1025],
                    ptail[:, 0:8].rearrange("p (s d) -> p s d", d=1))
                nc.gpsimd.memset(chunks3d[:, :, 1025:1026], 0.0)
                # stats: DVE chunks 0-2, ACT chunks 3-7 (abs+accum), mtail partial
                nc.vector.tensor_reduce(
                    acc[:, 0:3], attn_sb[:, 0:3 * AS].rearrange("p (s d) -> p s d", d=AS),
                    axis=mybir.AxisListType.X, op=mybir.AluOpType.add,
                    apply_absolute_value=True)
                # abs scratch lands in yt (quant overwrites it afterwards)
                nc.scalar.activation(
                    yt[:, 3 * AS:8 * AS], attn_sb[:, 3 * AS:8 * AS],
                    mybir.ActivationFunctionType.Abs, accum_out=acc[:, 3:4])
                # fold this head's m-tail partial in at partition 0, then
                # all-reduce across partitions on Pool (keeps PE out of the chain)
                nc.vector.memset(acc[:, 4:5], 0.0)
                nc.vector.tensor_copy(acc[0:1, 4:5], mta_row[0:1, h:h + 1])
                nc.vector.tensor_reduce(
                    acc[:, 5:6], acc[:, 0:5],
                    axis=mybir.AxisListType.X, op=mybir.AluOpType.add)
                nc.gpsimd.partition_all_reduce(rho_b[:, 5:6], acc[:, 5:6],
                                               channels=128,
                                               reduce_op=bass_isa.ReduceOp.add)
                nc.vector.tensor_scalar(rho_b[:, 1:2], rho_b[:, 5:6],
                                        1.0 / (N * N), float(C_EPS),
                                        op0=mybir.AluOpType.mult,
                                        op1=mybir.AluOpType.add)
                nc.vector.reciprocal(rho_b[:, 0:1], rho_b[:, 1:2])
                nc.vector.tensor_scalar(rho_b[:, 2:3], rho_b[:, 1:2], float(KAPPA), None,
                                        op0=mybir.AluOpType.mult)
                # -192*colsum(v) bias for this head, partition-aligned to 0..63
                cvs = cvcol[b][(h % 2) * 64:(h % 2) * 64 + 64, h // 2:h // 2 + 1]
                nc.vector.tensor_scalar(rho_b[0:64, 3:4], cvs, -M192, None,
                                        op0=mybir.AluOpType.mult)
                # bias*kt for the ACT osb path
                nc.vector.tensor_scalar(rho_b[0:64, 4:5], rho_b[0:64, 3:4],
                                        rho_b[0:64, 2:3], None,
                                        op0=mybir.AluOpType.mult)

            def emit_quant(attn_sb, yt, ytail, mt_sb, rho_b):
                # y = clamp(bf16(a*rho + 192), 191, 193); groups (0-3), (4-7)
                for g in range(2):
                    sl_in = attn_sb[:, g * 4 * AS:(g + 1) * 4 * AS]
                    sl_out = yt[:, g * 4 * AS:(g + 1) * 4 * AS]
                    nc.vector.tensor_scalar(
                        sl_out, sl_in, rho_b[:, 0:1], M192,
                        op0=mybir.AluOpType.mult, op1=mybir.AluOpType.add)
                    if g == 0:
                        nc.vector.tensor_scalar(
                            sl_out, sl_out, 193.0, 191.0,
                            op0=mybir.AluOpType.min, op1=mybir.AluOpType.max)
                    else:
                        nc.vector.tensor_scalar(
                            yt[:, 4 * AS:6 * AS], yt[:, 4 * AS:6 * AS], 193.0, 191.0,
                            op0=mybir.AluOpType.min, op1=mybir.AluOpType.max)
                        nc.gpsimd.tensor_scalar(
                            yt[:, 6 * AS:8 * AS], yt[:, 6 * AS:8 * AS], 193.0, 191.0,
                            op0=mybir.AluOpType.min, op1=mybir.AluOpType.max)
                # m-tail row for this head
                nc.vector.tensor_scalar(
                    ytail[:], mt_sb[:], rho_b[0:12, 0:1], M192,
                    op0=mybir.AluOpType.mult, op1=mybir.AluOpType.add)
                nc.vector.tensor_scalar(
                    ytail[:], ytail[:], 193.0, 191.0,
                    op0=mybir.AluOpType.min, op1=mybir.AluOpType.max)

            def emit_av(b, h, yt, ytail, vtb, rho_b):
                pso = [ps_av.tile([128, 512], F32, tag="ps_av",
                                  name=f"pso_{b}_{h}_{i}") for i in range(2)]
                pst = ps_av.tile([128, 16], F32, tag="ps_av", name=f"psot_{b}_{h}")
                for mi, (m0, ms) in enumerate(M_FULL):
                    vsl = vq[b][:ms, mi * C + h * D:mi * C + h * D + D]
                    for ti, (n0, nt) in enumerate(NT2):
                        nc.tensor.matmul(
                            pso[ti][:64, :nt], vsl, yt[:ms, mi * AS + n0:mi * AS + n0 + nt],
                            start=(mi == 0), stop=False)
                    nc.tensor.matmul(
                        pst[:64, 0:1], vsl, yt[:ms, mi * AS + 1024:mi * AS + 1025],
                        start=(mi == 0), stop=False)
                # m-tail row contribution: one-hot-masked v-tail, 12-partition contract
                vmask = mt_p.tile([12, D], BF16, tag="vmask")
                nc.vector.tensor_scalar(vmask[:], vtb[0:12, h * D:(h + 1) * D],
                                        identf[0:12, h:h + 1], None,
                                        op0=mybir.AluOpType.mult)
                for ti, (n0, nt) in enumerate(NT2):
                    nc.tensor.matmul(pso[ti][:64, :nt], vmask[:], ytail[0:12, n0:n0 + nt],
                                     start=False, stop=True)
                nc.tensor.matmul(pst[:64, 0:1], vmask[:], ytail[0:12, 1024:1025],
                                 start=False, stop=True)
                return pso, pst

            def emit_out(b, h, pso, pst, rho_b):
                # out = (pso - 192*colsum_v) * kt
                kt_col = rho_b[0:64, 2:3]
                bias_col = rho_b[0:64, 3:4]
                biaskt_col = rho_b[0:64, 4:5]
                for ti, (n0, nt) in enumerate(NT2):
                    osb = os_p.tile([64, 512], F32, tag="outsb")
                    if ti == 0:
                        nc.vector.tensor_scalar(osb[:, :nt], pso[ti][:64, :nt],
                                                bias_col, kt_col,
                                                op0=mybir.AluOpType.add,
                                                op1=mybir.AluOpType.mult)
                    else:
                        nc.scalar.activation(osb[:, :nt], pso[ti][:64, :nt],
                                             mybir.ActivationFunctionType.Identity,
                                             scale=kt_col, bias=biaskt_col)
                    nc.sync.dma_start(y_d[b, h * D:(h + 1) * D, n0:n0 + nt], osb[:, :nt])
                osb = os_p.tile([64, 512], F32, tag="outsb")
                nc.vector.tensor_scalar(osb[:, 0:1], pst[:64, 0:1],
                                        bias_col, kt_col,
                                        op0=mybir.AluOpType.add,
                                        op1=mybir.AluOpType.mult)
                nc.sync.dma_start(y_d[b, h * D:(h + 1) * D, 1024:1025], osb[:, 0:1])

            for b in (range(BPC) if _PH in ("full", "B") else []):
                mt_sb, mta_row, vtb = emit_batch_prep(b)
                prev = None
                for h in range(H):
                    attn_sb = as_p.tile([128, 8 * AS], BF16, tag="attnsb")
                    yt = y_p.tile([128, 8 * AS], BF16, tag="ytile")
                    ytail = mt_p.tile([12, AS], BF16, tag="ytail")
                    acc = acc_p.tile([128, 6], F32, tag="acc")
                    rho_b = st_p.tile([128, 6], F32, tag="rho_b")
                    ptail = ps_tl.tile([128, 16], F32, tag="ps_tl")
                    qkt = emit_qk(b, h, ptail)
                    if prev is not None:
                        pv = prev
                        pso, pst = emit_av(b, pv["h"], pv["yt"], pv["ytail"], vtb, pv["rho_b"])
                        emit_out(b, pv["h"], pso, pst, pv["rho_b"])
                    emit_drain_stats(b, h, attn_sb, yt, qkt, ptail, acc, rho_b, mta_row)
                    emit_quant(attn_sb, yt, ytail, mt_sb, rho_b)
                    prev = {"h": h, "yt": yt, "ytail": ytail, "rho_b": rho_b}
                pv = prev
                pso, pst = emit_av(b, pv["h"], pv["yt"], pv["ytail"], vtb, pv["rho_b"])
                emit_out(b, pv["h"], pso, pst, pv["rho_b"])
    nc.finalize()
    return nc


_NC = None

def _get_nc():
    global _NC
    if _NC is None:
        _NC = build_nc()
    return _NC


def _make_in_maps(x, w_qkv):
    x = np.ascontiguousarray(x, dtype=np.float32)
    w = np.ascontiguousarray(w_qkv, dtype=np.float32)
    s_w = np.float32(np.mean(np.abs(w)) + np.float32(EPS))
    wq_int = np.round(np.clip(w / s_w, -1, 1)).astype(np.float32)  # [3C, C]
    wt = np.ascontiguousarray(wq_int.T).astype(ml_dtypes.bfloat16)  # [C, 3C]
    ident = np.eye(128, dtype=ml_dtypes.bfloat16)
    ones128 = np.ones((128, 1), np.float32)

    xt = x.transpose(0, 2, 1)  # [B, C, N]
    xh = xt.astype(ml_dtypes.bfloat16)
    xl = (xt - xh.astype(np.float32)).astype(ml_dtypes.bfloat16)

    in_maps = []
    for core in range(8):
        sl = slice(core * BPC, (core + 1) * BPC)
        in_maps.append({
            "x_hi": np.ascontiguousarray(xh[sl]),
            "x_lo": np.ascontiguousarray(xl[sl]),
            "wt_bf": wt, "ident": ident, "ones128": ones128,
        })
    return in_maps


def kernel(x, w_qkv):
    in_maps = _make_in_maps(x, w_qkv)
    nc = _get_nc()
    res = run_bass_kernel_spmd(nc, in_maps, core_ids=list(range(8)))
    out = np.empty((B, N, C), np.float32)
    for core in range(8):
        out[core * BPC:(core + 1) * BPC] = res.results[core]["y_sh"].transpose(0, 2, 1)
    return out


# revision 55
# speedup vs baseline: 1.3622x; 1.0358x over previous
"""Trainium2 Bass kernel for ternary-quantized attention (BitNet-style).

Host contract: kernel(x, w_qkv) -> [16,1025,768] fp32.
Shards B=16 over 8 cores (2 batches/core), replicates the ternary weight.

Math (matches fp32 reference to ~0.8% rel err):
  - w ternarized on host to {-1,0,1}; s_w folded out (scale-invariant l1norm).
  - qkv = x @ wt with x as fp32r stationary, wt bf16 moving (exact products,
    fp32 PSUM accumulate).
  - q/k/v quantize: u = t / (l1_row * s_const), s_const = 1/64 + 1e-5;
    ternary = sign(bf16(u + 192) - 192)   [bf16 write rounds to int, RNE]
  - attn_int = q_q @ k_q^T (exact ternary bf16 matmuls, fp32 accum)
  - per-(b,h) scale: t = mean|attn_int| + EPS/(scale*s_const^2); rho = 1/t
  - y = clamp(bf16(attn_int*rho + 192), 191, 193) in {191,192,193}
  - out = (y @ v_q - 192*colsum(v_q)) * (scale * s_const^3 * t)
    [-192*colsum folded in as a rank-1 correction matmul into PSUM]
  - the m=1024 attn row is computed for all 12 heads at once via a
    block-diagonal k-tail stationary matrix (per batch, not per head).
"""
import sys, os
sys.path.insert(0, "/opt/trn_rl_repo")
import numpy as np
import ml_dtypes
from contextlib import ExitStack

import concourse.bass as bass
import concourse.tile as tile
from concourse import bacc
from concourse import mybir
from concourse import bass_isa
from concourse.bass_utils import run_bass_kernel_spmd

EPS = 1e-5
B, N, C, H, D = 16, 1025, 768, 12, 64
BPC = B // 8  # batches per core
SCALE = float(D) ** -0.5
S_CONST = np.float32(1.0 / D) + np.float32(EPS)
C_EPS = np.float32(EPS) / (np.float32(SCALE) * S_CONST * S_CONST)
KAPPA = np.float32(SCALE) * S_CONST * S_CONST * S_CONST
M192 = 192.0

F32 = mybir.dt.float32
F32R = mybir.dt.float32r
BF16 = mybir.dt.bfloat16

N_CHUNKS = [(i * 128, 128) for i in range(8)] + [(1024, 1)]
M_FULL = [(i * 128, 128) for i in range(8)]  # m-tail row handled batched
AS = 1026  # attn_sb per-m-chunk column stride (even)
QKV_TILES = [(0, 512), (512, 512), (1024, 512), (1536, 512), (2048, 256)]
NT2 = ((0, 512), (512, 512))


def build_nc():
    nc = bacc.Bacc("TRN2", target_bir_lowering=False, debug=False,
                   enable_asserts=False, num_devices=8)
    for val in (-M192, M192):
        t = nc.alloc_sbuf_tensor(f"const-f32-{val}", [128, 1], F32)
        nc.gpsimd.memset(t.ap(), val)
        nc.const_aps.aps[(F32, val)] = t.ap()
    nc.all_engine_barrier()
    xh_d = nc.dram_tensor("x_hi", [BPC, C, N], BF16, kind="ExternalInput").ap()
    xl_d = nc.dram_tensor("x_lo", [BPC, C, N], BF16, kind="ExternalInput").ap()
    wt_d = nc.dram_tensor("wt_bf", [C, 3 * C], BF16, kind="ExternalInput").ap()
    qt_d = nc.dram_tensor("qkvt", [BPC, 3 * C], F32, kind="ExternalInput").ap()
    id_d = nc.dram_tensor("ident", [128, 128], BF16, kind="ExternalInput").ap()
    on_d = nc.dram_tensor("ones128", [128, 1], F32, kind="ExternalInput").ap()
    y_d = nc.dram_tensor("y_sh", [BPC, C, N], F32, kind="ExternalOutput").ap()

    with tile.TileContext(nc) as tc, ExitStack() as ctx:
        const_p = ctx.enter_context(tc.tile_pool(name="consts", bufs=1))
        qt_p = ctx.enter_context(tc.tile_pool(name="qt", bufs=6 * BPC))
        kt_p = ctx.enter_context(tc.tile_pool(name="kt", bufs=6 * BPC))
        vq_p = ctx.enter_context(tc.tile_pool(name="vq", bufs=BPC))
        cv_p = ctx.enter_context(tc.tile_pool(name="cv", bufs=BPC))

        ident = const_p.tile([128, 128], BF16, tag="ident")
        nc.sync.dma_start(ident[:], id_d)
        ones128 = const_p.tile([128, 1], F32, tag="ones")
        nc.sync.dma_start(ones128[:], on_d)
        ones128b = const_p.tile([128, 1], BF16, tag="onesb")
        nc.vector.tensor_copy(ones128b[:], ones128[:])
        identf = const_p.tile([128, 128], F32, tag="identf")
        nc.vector.tensor_copy(identf[:], ident[:])

        qT = [[qt_p.tile([128, N], BF16, tag="qt", name=f"qT_{b}_{j}") for j in range(6)] for b in range(BPC)]
        kT = [[kt_p.tile([128, N], BF16, tag="kt", name=f"kT_{b}_{j}") for j in range(6)] for b in range(BPC)]
        vq = [vq_p.tile([128, 9 * C], BF16, tag="vq", name=f"vq_{b}") for b in range(BPC)]
        # colsum(vq) as columns: [128, 6]; col j = c-chunk j (2 heads stacked)
        cvcol = [cv_p.tile([128, 8], F32, tag="cv", name=f"cv_{b}") for b in range(BPC)]

        _PH = os.environ.get("KERNEL_PHASE", "full")
        # ================= PHASE A: qkv + quantize + transpose =================
        with tc.tile_pool(name="wt", bufs=6) as wt_p, \
             tc.tile_pool(name="xs", bufs=6) as xs_p, \
             tc.tile_pool(name="qkvsb", bufs=2) as qkvsb_p, \
             tc.tile_pool(name="small_a", bufs=4) as small_p, \
             tc.tile_pool(name="y192", bufs=2) as y192_p, \
             tc.tile_pool(name="qkq", bufs=2) as qkq_p, \
             tc.tile_pool(name="ps_qkv", bufs=5, space="PSUM") as ps_qkv, \
             tc.tile_pool(name="ps_tr", bufs=3, space="PSUM") as ps_tr:
            wt = []
            for c in range(6):
                t = wt_p.tile([128, 3 * C], BF16, tag="wt")
                nc.sync.dma_start(t[:], wt_d[c * 128:(c + 1) * 128, :])
                wt.append(t)

            pend_tr = []

            def emit_transposes(qkq_t, b, n0, ns):
                for j in range(12):
                    pt = ps_tr.tile([128, 128], BF16, tag="ps_tr")
                    nc.tensor.transpose(pt[:, :ns], qkq_t[:ns, j * 128:(j + 1) * 128],
                                        ident[:ns, :ns])
                    dst = qT[b][j] if j < 6 else kT[b][j - 6]
                    if j % 2 == 0:
                        nc.vector.tensor_copy(dst[:, n0:n0 + ns], pt[:, :ns])
                    else:
                        nc.scalar.copy(dst[:, n0:n0 + ns], pt[:, :ns])

            for b in (range(BPC) if _PH in ("full", "A") else []):
                xs = []
                for c in range(6):
                    xh = xs_p.tile([128, N], BF16, tag="xh")
                    nc.sync.dma_start(xh[:], xh_d[b, c * 128:(c + 1) * 128, :])
                    xl = xs_p.tile([128, N], BF16, tag="xl")
                    nc.sync.dma_start(xl[:], xl_d[b, c * 128:(c + 1) * 128, :])
                    xs.append((xh, xl))

                for nci, (n0, ns) in enumerate(N_CHUNKS):
                    qkv_sb = qkvsb_p.tile([128, 3 * C], F32, tag="qkvsb")
                    if ns == 1:
                        # n=1024 row: qkv precomputed on host (exact fp32)
                        nc.sync.dma_start(qkv_sb[0:1, :], qt_d[b:b + 1, :])
                    else:
                        pss = []
                        for ti, (o0, osz) in enumerate(QKV_TILES):
                            ps = ps_qkv.tile([128, 512], F32, tag="ps_qkv")
                            nmm = 0
                            for c in range(6):
                                for xop in xs[c]:
                                    nc.tensor.matmul(
                                        ps[:ns, :osz],
                                        xop[:, n0:n0 + ns],
                                        wt[c][:, o0:o0 + osz],
                                        start=(nmm == 0), stop=(nmm == 11))
                                    nmm += 1
                            pss.append((ps, o0, osz))
                        for ti, (ps, o0, osz) in enumerate(pss):
                            if ti in (0, 2):
                                nc.vector.tensor_copy(qkv_sb[:ns, o0:o0 + osz], ps[:ns, :osz])
                            else:
                                nc.scalar.copy(qkv_sb[:ns, o0:o0 + osz], ps[:ns, :osz])
                    # l1 over D-segments: [ns, 36] (DVE; gpsimd can't free-reduce)
                    l1 = small_p.tile([128, 36], F32, tag="l1")
                    nc.vector.tensor_reduce(
                        l1[:ns, :], qkv_sb[:ns, :].rearrange("p (s d) -> p s d", d=D),
                        axis=mybir.AxisListType.X, op=mybir.AluOpType.add,
                        apply_absolute_value=True)
                    rho = small_p.tile([128, 36], F32, tag="rho")
                    nc.vector.tensor_scalar(l1[:ns, :], l1[:ns, :], float(S_CONST), None,
                                            op0=mybir.AluOpType.mult)
                    nc.vector.reciprocal(rho[:ns, :], l1[:ns, :])
                    # u*rho + 192 -> bf16 (rounds); 36 segs split DVE/Pool/ACT
                    y192 = y192_p.tile([128, 3 * C], BF16, tag="y192")
                    for s in range(36):
                        sl_in = qkv_sb[:ns, s * D:(s + 1) * D]
                        sl_out = y192[:ns, s * D:(s + 1) * D]
                        r = s % 3
                        if r == 0:
                            nc.vector.tensor_scalar(
                                sl_out, sl_in, rho[:ns, s:s + 1], M192,
                                op0=mybir.AluOpType.mult, op1=mybir.AluOpType.add)
                        elif r == 1:
                            nc.gpsimd.tensor_scalar(
                                sl_out, sl_in, rho[:ns, s:s + 1], M192,
                                op0=mybir.AluOpType.mult, op1=mybir.AluOpType.add)
                        else:
                            nc.scalar.activation(
                                sl_out, sl_in, mybir.ActivationFunctionType.Identity,
                                bias=M192, scale=rho[:ns, s:s + 1])
                    # q,k ternary via ACT Sign -> qkq
                    qkq = qkq_p.tile([128, 2 * C], BF16, tag="qkq")
                    nc.scalar.activation(qkq[:ns, :], y192[:ns, :2 * C],
                                         mybir.ActivationFunctionType.Sign, bias=-M192)
                    # v ternary via DVE clamp + sub -> vq
                    nc.vector.tensor_scalar(
                        y192[:ns, 2 * C:], y192[:ns, 2 * C:], 193.0, 191.0,
                        op0=mybir.AluOpType.min, op1=mybir.AluOpType.max)
                    nc.vector.tensor_scalar(
                        vq[b][:ns, nci * C:(nci + 1) * C], y192[:ns, 2 * C:],
                        M192, None, op0=mybir.AluOpType.subtract)
                    if pend_tr:
                        emit_transposes(*pend_tr.pop())
                    pend_tr.append((qkq, b, n0, ns))
                if pend_tr:
                    emit_transposes(*pend_tr.pop())

        # ================= PHASE B: attention =================
        with tc.tile_pool(name="attnsb", bufs=2) as as_p, \
             tc.tile_pool(name="ytile", bufs=2) as y_p, \
             tc.tile_pool(name="mt", bufs=2) as mt_p, \
             tc.tile_pool(name="acc", bufs=2) as acc_p, \
             tc.tile_pool(name="stat", bufs=4) as st_p, \
             tc.tile_pool(name="outsb", bufs=3) as os_p, \
             tc.tile_pool(name="ps_qk", bufs=2, space="PSUM") as ps_qk, \
             tc.tile_pool(name="ps_av", bufs=3, space="PSUM") as ps_av, \
             tc.tile_pool(name="ps_tl", bufs=1, space="PSUM") as ps_tl:
            def emit_batch_prep(b):
                """colsum(vq), batched m=1024 attn rows for all heads, vtail bcast."""
                # --- colsum of vq, column-oriented: out [128, 6], col j = c-chunk ---
                cvps = ps_tl.tile([128, 16], F32, tag="ps_tl", name=f"cvps_{b}")
                for cc in range(6):
                    for mi in range(9):
                        ms = 128 if mi < 8 else 1
                        nc.tensor.matmul(
                            cvps[:, cc:cc + 1],
                            vq[b][:ms, mi * C + cc * 128:mi * C + (cc + 1) * 128],
                            ones128b[:ms, :],
                            start=(mi == 0), stop=(mi == 8))
                nc.vector.tensor_copy(cvcol[b][:, 0:6], cvps[:, 0:6])
                # --- batched m-tail: attn[1024, n] for all 12 heads ---
                ktd = mt_p.tile([128, 72], BF16, tag="ktd", name=f"ktd_{b}")
                nc.vector.memset(ktd[:], 0.0)
                for h in range(H):
                    j, r0 = h // 2, (h % 2) * 64
                    nc.vector.tensor_copy(ktd[r0:r0 + 64, j * 12 + h:j * 12 + h + 1],
                                          kT[b][j][r0:r0 + 64, 1024:1025])
                mtps = ps_qk.tile([128, 1024], F32, tag="ps_qk", name=f"mtps_{b}")
                mtt = ps_tl.tile([128, 16], F32, tag="ps_tl", name=f"mtt_{b}")
                for j in range(6):
                    for (n0, nt) in NT2:
                        nc.tensor.matmul(
                            mtps[0:12, n0:n0 + nt],
                            ktd[:, j * 12:(j + 1) * 12],
                            qT[b][j][:, n0:n0 + nt],
                            start=(j == 0), stop=(j == 5))
                    # attn[1024,1024] for all heads -> [12, 1]
                    nc.tensor.matmul(
                        mtt[0:12, 0:1],
                        ktd[:, j * 12:(j + 1) * 12],
                        qT[b][j][:, 1024:1025],
                        start=(j == 0), stop=(j == 5))
                mt_sb = mt_p.tile([12, AS], BF16, tag="mt", name=f"mt_{b}")
                nc.scalar.copy(mt_sb[:, 0:1024], mtps[0:12, 0:1024])
                nc.vector.tensor_copy(mt_sb[:, 1024:1025], mtt[0:12, 0:1])
                # per-head |.| partials over the m-tail row: [12, 1]
                mta = mt_p.tile([12, 1], F32, tag="mta", name=f"mta_{b}")
                nc.vector.tensor_reduce(
                    mta[:], mt_sb[:, 0:1025],
                    axis=mybir.AxisListType.X, op=mybir.AluOpType.add,
                    apply_absolute_value=True)
                # transpose to a [1, 12] row so per-head scalars are partition-0
                nc.tensor.matmul(mtt[0:1, 2:14], mta[:], identf[0:12, 0:12],
                                 start=True, stop=True)
                mta_row = mt_p.tile([1, 12], F32, tag="mtarow", name=f"mtarow_{b}")
                nc.vector.tensor_copy(mta_row[:], mtt[0:1, 2:14])
                # v tail row broadcast to partitions 0..11 for per-head AV
                vtb = mt_p.tile([12, C], BF16, tag="vtb", name=f"vtb_{b}")
                nc.gpsimd.partition_broadcast(vtb[:], vq[b][0:1, 8 * C:9 * C])
                return mt_sb, mta_row, vtb

            def emit_qk(b, h, ptail):
                j, r0 = h // 2, (h % 2) * 64
                qkt = []
                for mi, (m0, ms) in enumerate(M_FULL):
                    ps = ps_qk.tile([128, 1024], F32, tag="ps_qk")
                    for (n0, nt) in NT2:
                        nc.tensor.matmul(
                            ps[:ms, n0:n0 + nt],
                            kT[b][j][r0:r0 + 64, m0:m0 + ms],
                            qT[b][j][r0:r0 + 64, n0:n0 + nt],
                            start=True, stop=True)
                    nc.tensor.matmul(
                        ptail[:ms, mi:mi + 1],
                        kT[b][j][r0:r0 + 64, m0:m0 + ms],
                        qT[b][j][r0:r0 + 64, 1024:1025],
                        start=True, stop=True)
                    qkt.append((ps, mi, ms))
                return qkt

            def emit_drain_stats(b, h, attn_sb, yt, qkt, ptail, acc, rho_b, mta_row):
                for (ps, mi, ms) in qkt:
                    dst = attn_sb[:ms, mi * AS:mi * AS + 1024]
                    if mi in (0, 2):
                        nc.vector.tensor_copy(dst, ps[:ms, :1024])
                    else:
                        nc.scalar.copy(dst, ps[:ms, :1024])
                chunks3d = attn_sb[:].rearrange("p (s d) -> p s d", d=AS)
                nc.vector.tensor_copy(
                    chunks3d[:, :, 1024:1025],
                    ptail[:, 0:8].rearrange("p (s d) -> p s d", d=1))

                # stats: DVE chunks 0-2, ACT chunks 3-7 (abs+accum), mtail partial
                nc.vector.tensor_reduce(
                    acc[:, 0:3], chunks3d[:, 0:3, 0:1025],
                    axis=mybir.AxisListType.X, op=mybir.AluOpType.add,
                    apply_absolute_value=True)
                # abs scratch lands in yt (quant overwrites it afterwards)
                yt3d = yt[:].rearrange("p (s d) -> p s d", d=AS)
                nc.scalar.activation(
                    yt3d[:, 3:8, 0:1025], chunks3d[:, 3:8, 0:1025],
                    mybir.ActivationFunctionType.Abs, accum_out=acc[:, 3:4])
                # fold this head's m-tail partial in at partition 0, then
                # all-reduce across partitions on Pool (keeps PE out of the chain)
                nc.vector.memset(acc[:, 4:5], 0.0)
                nc.vector.tensor_copy(acc[0:1, 4:5], mta_row[0:1, h:h + 1])
                nc.vector.tensor_reduce(
                    acc[:, 5:6], acc[:, 0:5],
                    axis=mybir.AxisListType.X, op=mybir.AluOpType.add)
                nc.gpsimd.partition_all_reduce(rho_b[:, 5:6], acc[:, 5:6],
                                               channels=128,
                                               reduce_op=bass_isa.ReduceOp.add)
                nc.vector.tensor_scalar(rho_b[:, 1:2], rho_b[:, 5:6],
                                        1.0 / (N * N), float(C_EPS),
                                        op0=mybir.AluOpType.mult,
                                        op1=mybir.AluOpType.add)
                nc.vector.reciprocal(rho_b[:, 0:1], rho_b[:, 1:2])
                nc.vector.tensor_scalar(rho_b[:, 2:3], rho_b[:, 1:2], float(KAPPA), None,
                                        op0=mybir.AluOpType.mult)
                # -192*colsum(v) bias for this head, partition-aligned to 0..63
                cvs = cvcol[b][(h % 2) * 64:(h % 2) * 64 + 64, h // 2:h // 2 + 1]
                nc.vector.tensor_scalar(rho_b[0:64, 3:4], cvs, -M192, None,
                                        op0=mybir.AluOpType.mult)
                # bias*kt for the ACT osb path
                nc.vector.tensor_scalar(rho_b[0:64, 4:5], rho_b[0:64, 3:4],
                                        rho_b[0:64, 2:3], None,
                                        op0=mybir.AluOpType.mult)

            def emit_quant(attn_sb, yt, ytail, mt_sb, rho_b):
                # y = clamp(bf16(a*rho + 192), 191, 193); groups (0-3), (4-7)
                for g in range(2):
                    sl_in = attn_sb[:, g * 4 * AS:(g + 1) * 4 * AS]
                    sl_out = yt[:, g * 4 * AS:(g + 1) * 4 * AS]
                    nc.vector.tensor_scalar(
                        sl_out, sl_in, rho_b[:, 0:1], M192,
                        op0=mybir.AluOpType.mult, op1=mybir.AluOpType.add)
                    if g == 0:
                        nc.vector.tensor_scalar(
                            sl_out, sl_out, 193.0, 191.0,
                            op0=mybir.AluOpType.min, op1=mybir.AluOpType.max)
                    else:
                        nc.vector.tensor_scalar(
                            yt[:, 4 * AS:6 * AS], yt[:, 4 * AS:6 * AS], 193.0, 191.0,
                            op0=mybir.AluOpType.min, op1=mybir.AluOpType.max)
                        nc.gpsimd.tensor_scalar(
                            yt[:, 6 * AS:8 * AS], yt[:, 6 * AS:8 * AS], 193.0, 191.0,
                            op0=mybir.AluOpType.min, op1=mybir.AluOpType.max)
                # m-tail row for this head
                nc.vector.tensor_scalar(
                    ytail[:], mt_sb[:], rho_b[0:12, 0:1], M192,
                    op0=mybir.AluOpType.mult, op1=mybir.AluOpType.add)
                nc.vector.tensor_scalar(
                    ytail[:], ytail[:], 193.0, 191.0,
                    op0=mybir.AluOpType.min, op1=mybir.AluOpType.max)

            def emit_av(b, h, yt, ytail, vtb, rho_b):
                pso = [ps_av.tile([128, 512], F32, tag="ps_av",
                                  name=f"pso_{b}_{h}_{i}") for i in range(2)]
                pst = ps_av.tile([128, 16], F32, tag="ps_av", name=f"psot_{b}_{h}")
                for mi, (m0, ms) in enumerate(M_FULL):
                    vsl = vq[b][:ms, mi * C + h * D:mi * C + h * D + D]
                    for ti, (n0, nt) in enumerate(NT2):
                        nc.tensor.matmul(
                            pso[ti][:64, :nt], vsl, yt[:ms, mi * AS + n0:mi * AS + n0 + nt],
                            start=(mi == 0), stop=False)
                    nc.tensor.matmul(
                        pst[:64, 0:1], vsl, yt[:ms, mi * AS + 1024:mi * AS + 1025],
                        start=(mi == 0), stop=False)
                # m-tail row contribution: one-hot-masked v-tail, 12-partition contract
                vmask = mt_p.tile([12, D], BF16, tag="vmask")
                nc.vector.tensor_scalar(vmask[:], vtb[0:12, h * D:(h + 1) * D],
                                        identf[0:12, h:h + 1], None,
                                        op0=mybir.AluOpType.mult)
                for ti, (n0, nt) in enumerate(NT2):
                    nc.tensor.matmul(pso[ti][:64, :nt], vmask[:], ytail[0:12, n0:n0 + nt],
                                     start=False, stop=True)
                nc.tensor.matmul(pst[:64, 0:1], vmask[:], ytail[0:12, 1024:1025],
                                 start=False, stop=True)
                return pso, pst

            def emit_out(b, h, pso, pst, rho_b):
                # out = (pso - 192*colsum_v) * kt
                kt_col = rho_b[0:64, 2:3]
                bias_col = rho_b[0:64, 3:4]
                biaskt_col = rho_b[0:64, 4:5]
                for ti, (n0, nt) in enumerate(NT2):
                    osb = os_p.tile([64, 512], F32, tag="outsb")
                    if ti == 0:
                        nc.vector.tensor_scalar(osb[:, :nt], pso[ti][:64, :nt],
                                                bias_col, kt_col,
                                                op0=mybir.AluOpType.add,
                                                op1=mybir.AluOpType.mult)
                    else:
                        nc.scalar.activation(osb[:, :nt], pso[ti][:64, :nt],
                                             mybir.ActivationFunctionType.Identity,
                                             scale=kt_col, bias=biaskt_col)
                    nc.sync.dma_start(y_d[b, h * D:(h + 1) * D, n0:n0 + nt], osb[:, :nt])
                osb = os_p.tile([64, 512], F32, tag="outsb")
                nc.vector.tensor_scalar(osb[:, 0:1], pst[:64, 0:1],
                                        bias_col, kt_col,
                                        op0=mybir.AluOpType.add,
                                        op1=mybir.AluOpType.mult)
                nc.sync.dma_start(y_d[b, h * D:(h + 1) * D, 1024:1025], osb[:, 0:1])

            for b in (range(BPC) if _PH in ("full", "B") else []):
                mt_sb, mta_row, vtb = emit_batch_prep(b)
                prev = None
                for h in range(H):
                    attn_sb = as_p.tile([128, 8 * AS], BF16, tag="attnsb")
                    yt = y_p.tile([128, 8 * AS], BF16, tag="ytile")
                    ytail = mt_p.tile([12, AS], BF16, tag="ytail")
                    acc = acc_p.tile([128, 6], F32, tag="acc")
                    rho_b = st_p.tile([128, 6], F32, tag="rho_b")
                    ptail = ps_tl.tile([128, 16], F32, tag="ps_tl")
                    qkt = emit_qk(b, h, ptail)
                    if prev is not None:
                        pv = prev
                        pso, pst = emit_av(b, pv["h"], pv["yt"], pv["ytail"], vtb, pv["rho_b"])
                        emit_out(b, pv["h"], pso, pst, pv["rho_b"])
                    emit_drain_stats(b, h, attn_sb, yt, qkt, ptail, acc, rho_b, mta_row)
                    emit_quant(attn_sb, yt, ytail, mt_sb, rho_b)
                    prev = {"h": h, "yt": yt, "ytail": ytail, "rho_b": rho_b}
                pv = prev
                pso, pst = emit_av(b, pv["h"], pv["yt"], pv["ytail"], vtb, pv["rho_b"])
                emit_out(b, pv["h"], pso, pst, pv["rho_b"])
    nc.finalize()
    return nc


_NC = None

def _get_nc():
    global _NC
    if _NC is None:
        _NC = build_nc()
    return _NC


def _make_in_maps(x, w_qkv):
    x = np.ascontiguousarray(x, dtype=np.float32)
    w = np.ascontiguousarray(w_qkv, dtype=np.float32)
    s_w = np.float32(np.mean(np.abs(w)) + np.float32(EPS))
    wq_int = np.round(np.clip(w / s_w, -1, 1)).astype(np.float32)  # [3C, C]
    wt = np.ascontiguousarray(wq_int.T).astype(ml_dtypes.bfloat16)  # [C, 3C]
    ident = np.eye(128, dtype=ml_dtypes.bfloat16)
    ones128 = np.ones((128, 1), np.float32)

    xt = x.transpose(0, 2, 1)  # [B, C, N]
    xh = xt.astype(ml_dtypes.bfloat16)
    xl = (xt - xh.astype(np.float32)).astype(ml_dtypes.bfloat16)
    # host-computed qkv for the n=1024 tail row (exact fp32, ternary w folded)
    qkvt = (x[:, 1024, :] @ wq_int.T).astype(np.float32)  # [B, 3C]

    in_maps = []
    for core in range(8):
        sl = slice(core * BPC, (core + 1) * BPC)
        in_maps.append({
            "x_hi": np.ascontiguousarray(xh[sl]),
            "x_lo": np.ascontiguousarray(xl[sl]),
            "qkvt": np.ascontiguousarray(qkvt[sl]),
            "wt_bf": wt, "ident": ident, "ones128": ones128,
        })
    return in_maps


def kernel(x, w_qkv):
    in_maps = _make_in_maps(x, w_qkv)
    nc = _get_nc()
    res = run_bass_kernel_spmd(nc, in_maps, core_ids=list(range(8)))
    out = np.empty((B, N, C), np.float32)
    for core in range(8):
        out[core * BPC:(core + 1) * BPC] = res.results[core]["y_sh"].transpose(0, 2, 1)
    return out
